# revision 24
# baseline (speedup 1.0000x reference)
"""GRU-ODE delay cell on 8 Trainium2 NeuronCores (Bass/Tile), fp8 DoubleRow.

Math (per reference):
    x   = x_coeffs[int(t)]                  # [B, I]
    r   = sigmoid([x, h] @ W_r.T)
    z   = sigmoid([x, h] @ W_z.T)
    h~  = tanh([x, r*h] @ W_h.T)
    dh  = (1 - z) * (h~ - h)

Strategy: data-parallel over batch (B=8192 -> 1024 rows/core), weights
replicated, transposed ([feature, batch]) layout throughout.

Precision plan (validated against the reference in fp64 sim):
  - r gate h-part and h~ gate rh-part run as fp8e4 DoubleRow matmuls
    (2 contraction rows per PE cell -> ~1.8x matmul throughput).
  - z gate and all x-parts run in fp16 (same PE speed as bf16, 10-bit
    mantissa): dh = (1-z)(h~-h) amplifies z errors by |h~-h| (up to ~6),
    so z cannot take fp8; fp16 makes its error negligible.
  - all weights are pre-scaled by 1024 on host; activations keep natural
    scale; every PSUM readout applies scale=1/1024 inside the ACT op.
    (fp8e4 min normal is 2^-6: scaling weights up moves their mass out
    of the subnormal range.)
  - final (h~ - h) subtraction uses fp32 h.
  Simulated max-rel-err 0.0158 vs tolerance 2e-2.

Orientation per core (hidden tile m of 128 rows, batch free dim 1024):
    psum[m, b] += W.T[k_tile, m_slice].T @ act.T[k_tile, b]
    fp16 stages: one 128-row k-subtile per matmul
    fp8 stages:  DoubleRow pair = 2 k-subtiles per matmul via 3D AP
                 [128, 2, cols]
"""

import numpy as np
import ml_dtypes

B, H, I, TMAX = 8192, 1024, 128, 128
NCORES = 8
BC = B // NCORES          # batch rows per core
NT = H // 128             # 8 hidden output tiles
MM_N = 512                # moving free-dim per matmul (one PSUM bank of fp32)
WS = 1024.0               # host-side weight pre-scale (exact power of 2)

# per-gate count of h-side k-subtiles (of 8) computed in fp8 DoubleRow;
# must be even. Rest (and the x subtile) run fp16.
NR_F8 = 8
NZ_F8 = 2
NH_F8 = 8

_F16 = np.float16
_F8 = ml_dtypes.float8_e4m3   # IEEE-ish variant, max +-240 == TRN FP8_EXP4

_cache = {}


def _build_nc():
    import concourse.bacc as bacc
    import concourse.tile as tile
    import concourse.mybir as mybir

    f32 = mybir.dt.float32
    f16 = mybir.dt.float16
    f8 = mybir.dt.float8e4
    AF = mybir.ActivationFunctionType
    DR = mybir.MatmulPerfMode.DoubleRow
    INV = 1.0 / WS

    nc = bacc.Bacc(
        "TRN2",
        target_bir_lowering=False,
        debug=False,
        enable_asserts=False,
        num_devices=NCORES,
    )

    # DRAM layouts mirror the SBUF tile shapes exactly (host pre-packs).
    xT_d = nc.dram_tensor("xT", [128, BC], f16, kind="ExternalInput").ap()
    h16_d = nc.dram_tensor("hT16", [128, 8, BC], f16, kind="ExternalInput").ap()
    h8_d = nc.dram_tensor("hT8", [128, 8, BC], f8, kind="ExternalInput").ap()
    wrx_d = nc.dram_tensor("wrx", [128, H], f16, kind="ExternalInput").ap()
    # wr8 chunked by output-column group (chunk c = all 8 k-subtiles for 256
    # consecutive gate columns) so r matmuls can start after one chunk.
    wr8_d = nc.dram_tensor("wr8", [4, 128, 8, 256], f8, kind="ExternalInput").ap()
    wz_d = nc.dram_tensor("wz", [128, 9, H], f16, kind="ExternalInput").ap()
    wz8_d = nc.dram_tensor("wz8", [128, 2, H], f8, kind="ExternalInput").ap()
    whx_d = nc.dram_tensor("whx", [128, H], f16, kind="ExternalInput").ap()
    wh8_d = nc.dram_tensor("wh8", [128, 8, H], f8, kind="ExternalInput").ap()
    dh_d = nc.dram_tensor("dhT", [NT, 128, BC], f16, kind="ExternalOutput").ap()
    # sink for the PE warm-up matmuls (keeps them from being DCE'd)
    warm_d = nc.dram_tensor("warm", [128, 4], f32, kind="ExternalOutput").ap()

    bhalves = [(j * MM_N, MM_N) for j in range(BC // MM_N)]

    with tile.TileContext(nc) as tc:
        with (
            tc.tile_pool(name="res", bufs=1) as res,
            tc.tile_pool(name="work", bufs=3) as work,
            tc.tile_pool(name="psum", bufs=4, space="PSUM") as psum,
        ):
            # ---- PE warm-up input (memset must precede the warm matmuls) ----
            warm_in = res.tile([128, 512], f16, name="warm_in", tag="warm_in")
            nc.vector.memset(warm_in[:], 0.0)

            # ---- resident loads, issue-ordered by first use. dma_start
            # descriptor generation costs ~0.65us on the ISSUING engine and
            # serializes per engine; concurrent transfers share the ~330GB/s
            # DMA fabric. The r-gate critical prefix (x, wrx, h8, wr8 chunks)
            # goes on sync alone; everything else is interleaved into the
            # scalar engine's program between r-tile activations so its
            # transfers don't steal bandwidth from the prefix. ----
            x_sb = res.tile([128, BC], f16, name="x_sb", tag="x_sb")
            wrx_sb = res.tile([128, H], f16, name="wrx_sb", tag="wrx_sb")
            wr8_sb = [
                res.tile([128, 8, 256], f8, name=f"wr8_{c}", tag=f"wr8_{c}")
                for c in range(4)
            ]
            h8_sb = res.tile([128, 8, BC], f8, name="h8_sb", tag="h8_sb")
            h16_sb = res.tile([128, 8, BC], f16, name="h16_sb", tag="h16_sb")
            wz_sb = res.tile([128, 9, H], f16, name="wz_sb", tag="wz_sb")
            wz8_sb = res.tile([128, 2, H], f8, name="wz8_sb", tag="wz8_sb")
            whx_sb = res.tile([128, H], f16, name="whx_sb", tag="whx_sb")
            wh8_sb = res.tile([128, 8, H], f8, name="wh8_sb", tag="wh8_sb")

            # critical prefix split across sync/scalar/gpsimd so the
            # ~0.65us per-descriptor issue cost is paid in parallel
            nc.sync.dma_start(x_sb[:, 0:512], xT_d[:, 0:512])
            nc.sync.dma_start(wrx_sb[:, 0:512], wrx_d[:, 0:512])
            nc.sync.dma_start(x_sb[:, 512:1024], xT_d[:, 512:1024])
            nc.sync.dma_start(wrx_sb[:, 512:1024], wrx_d[:, 512:1024])
            for c in range(4):
                nc.sync.dma_start(wr8_sb[c][:], wr8_d[c])
            nc.gpsimd.dma_start(h8_sb[:, 0:2, :], h8_d[:, 0:2, :])
            nc.gpsimd.dma_start(h8_sb[:, 2:4, :], h8_d[:, 2:4, :])
            nc.gpsimd.dma_start(h8_sb[:, 4:6, :], h8_d[:, 4:6, :])
            nc.gpsimd.dma_start(h8_sb[:, 6:8, :], h8_d[:, 6:8, :])

            # ---- PE warm-up: keep the PE busy from t0 so the HAM clock
            # gate reaches 2.4 GHz before the first real matmul. ~13 warm-ups
            # bridge the ~8.5us DMA latency of the first loads. The
            # warm output DMA sits on gpsimd BEHIND the h8 issues so it can't
            # delay them.
            warm_ps = psum.tile([128, 512], f32, name="warm_ps", tag="ps")
            for _ in range(22):
                nc.tensor.matmul(
                    warm_ps[:], warm_in[:, :128], warm_in[:], start=True, stop=True
                )
            warm_sb = res.tile([128, 4], f32, name="warm_sb", tag="warm_sb")
            nc.vector.tensor_copy(warm_sb[:], warm_ps[:, :4])
            nc.gpsimd.dma_start(warm_d[:], warm_sb[:])

            # late loads, interleaved into the scalar program per r tile:
            # h16 (rh muls + z moving), wz (~22us in), wh (~28us in)
            late_loads = [
                [(h16_sb[:, 0:2, :], h16_d[:, 0:2, :]),
                 (h16_sb[:, 2:4, :], h16_d[:, 2:4, :])],
                [(h16_sb[:, 4:6, :], h16_d[:, 4:6, :]),
                 (h16_sb[:, 6:8, :], h16_d[:, 6:8, :]),
                 (whx_sb[:], whx_d[:])],
                [(wz8_sb[:], wz8_d[:]),
                 (wz_sb[:, 0:5, :], wz_d[:, 0:5, :])],
                [(wh8_sb[:], wh8_d[:]),
                 (wz_sb[:, 5:9, :], wz_d[:, 5:9, :])],
                [], [], [], [],
            ]

            rh8_sb = res.tile([128, 8, BC], f8, name="rh8_sb", tag="rh8_sb")
            rh16_sb = None
            if NH_F8 < 8:
                rh16_sb = res.tile(
                    [128, 8 - NH_F8, BC], f16, name="rh16_sb", tag="rh16_sb"
                )
            # zm persists only for the two z tiles computed before the h gate
            zm_sb = [
                res.tile([128, BC], f16, name=f"zm{k}", tag=f"zm{k}")
                for k in range(2)
            ]
            # d = (h~ - h) persists for tiles whose z gate runs last
            d_sb = [
                res.tile([128, BC], f16, name=f"d{k}", tag=f"d{k}")
                for k in range(2, NT)
            ]

            def gate_x(ps, n, wx, wz16, halves=None, ps_off=0):
                """x-part stage (fp16, always first -> start=True)."""
                cols = slice(n * 128, (n + 1) * 128)
                lhsT = wx[:, cols] if wx is not None else wz16[:, 0, cols]
                for b0, bw in halves or bhalves:
                    nc.tensor.matmul(
                        ps[:, b0 + ps_off : b0 + ps_off + bw],
                        lhsT,
                        x_sb[:, b0 : b0 + bw],
                        start=True,
                        stop=False,
                    )

            def gate_h(ps, n, w8, wz16, nf8, rhs8, rhs16, rhs16_off=0,
                       halves=None, ps_off=0):
                """h-part stages: nf8 k-subtiles as fp8 DoubleRow pairs,
                the rest fp16. Emitted after gate_x (start=False)."""
                cols = slice(n * 128, (n + 1) * 128)
                nstage = nf8 // 2 + (8 - nf8)
                stage = 0
                for p in range(nf8 // 2):
                    kk = slice(2 * p, 2 * p + 2)
                    stage += 1
                    if isinstance(w8, list):
                        off = (n % 2) * 128
                        lhsT8 = w8[n // 2][:, kk, off : off + 128]
                    else:
                        lhsT8 = w8[:, kk, cols]
                    for b0, bw in halves or bhalves:
                        nc.tensor.matmul(
                            ps[:, b0 + ps_off : b0 + ps_off + bw],
                            lhsT8,
                            rhs8[:, kk, b0 : b0 + bw],
                            start=False,
                            stop=(stage == nstage),
                            perf_mode=DR,
                        )
                for k in range(nf8, 8):
                    stage += 1
                    lhsT = wz16[:, k + 1, cols]
                    rhs = rhs16[:, k - rhs16_off, :]
                    for b0, bw in halves or bhalves:
                        nc.tensor.matmul(
                            ps[:, b0 + ps_off : b0 + ps_off + bw],
                            lhsT,
                            rhs[:, b0 : b0 + bw],
                            start=False,
                            stop=(stage == nstage),
                        )

            def gate_mms(ps, n, wx, w8, wz16, nf8, rhs8, rhs16, rhs16_off=0,
                         halves=None, ps_off=0):
                gate_x(ps, n, wx, wz16, halves, ps_off)
                gate_h(ps, n, w8, wz16, nf8, rhs8, rhs16, rhs16_off, halves,
                       ps_off)

            # ---- r gate ----
            # x-stages of the first 4 tiles run up front: x lands ~2us before
            # the first fp8 chunks, so this gives the PE ~1.7us of real work
            # to chew on while h8/wr8 stream in.
            ps_r = {}
            for n in range(4):
                ps_r[n] = psum.tile([128, BC], f32, name="ps_r", tag="ps")
                gate_x(ps_r[n], n, wrx_sb, None)
            for n in range(NT):
                if n in ps_r:
                    ps = ps_r[n]
                    gate_h(ps, n, wr8_sb, None, NR_F8, h8_sb, h16_sb)
                else:
                    ps = psum.tile([128, BC], f32, name="ps_r", tag="ps")
                    gate_mms(ps, n, wrx_sb, wr8_sb, None, NR_F8, h8_sb, h16_sb)
                r_t = work.tile([128, BC], f16, name="r_t", tag="r_t")
                nc.scalar.activation(r_t[:], ps[:], AF.Sigmoid, scale=INV)
                for dst, src in late_loads[n]:
                    nc.scalar.dma_start(dst, src)
                # rh = r * h (fp8 for DoubleRow stages, fp16 for the rest)
                if NH_F8 > 0:
                    nc.vector.tensor_mul(
                        rh8_sb[:, n, :], r_t[:], h16_sb[:, n, :]
                    )
                if NH_F8 < 8 and n >= NH_F8:
                    nc.vector.tensor_mul(
                        rh16_sb[:, n - NH_F8, :], r_t[:], h16_sb[:, n, :]
                    )

            # ---- z gate, first two tiles (store zm = 1 - z = sigmoid(-pre)),
            # giving the scalar/vector engines time to finish rh[7] ----
            for n in range(2):
                ps = psum.tile([128, BC], f32, name="ps_z", tag="ps")
                gate_mms(ps, n, None, wz8_sb, wz_sb, NZ_F8, h8_sb, h16_sb)
                nc.scalar.activation(zm_sb[n][:], ps[:], AF.Sigmoid, scale=-INV)

            # ---- candidate gate ----
            for n in range(NT):
                ps = psum.tile([128, BC], f32, name="ps_h", tag="ps")
                gate_mms(
                    ps, n, whx_sb, wh8_sb, wz_sb, NH_F8, rh8_sb, rh16_sb,
                    rhs16_off=NH_F8,
                )
                for b0, bw in bhalves:
                    sl = slice(b0, b0 + bw)
                    ht = work.tile([128, bw], f16, name="ht", tag="ht")
                    nc.scalar.activation(ht[:], ps[:, sl], AF.Tanh, scale=INV)
                    if n < 2:
                        # z already known: finish dh = zm * (h~ - h) now
                        d_t = work.tile([128, bw], f16, name="d_t", tag="d_t")
                        nc.vector.tensor_sub(d_t[:], ht[:], h16_sb[:, n, sl])
                        o_t = work.tile([128, bw], f16, name="o_t", tag="o_t")
                        nc.vector.tensor_mul(o_t[:], d_t[:], zm_sb[n][:, sl])
                        nc.sync.dma_start(dh_d[n][:, sl], o_t[:])
                    else:
                        # stash h~ - h; z for this tile is computed afterwards
                        nc.vector.tensor_sub(
                            d_sb[n - 2][:, sl], ht[:], h16_sb[:, n, sl]
                        )

            # ---- z gate, remaining tiles + output ----
            # ends the kernel on the short chain sigmoid -> mul -> DMA;
            # the final tile runs in 256-wide chunks to shorten the tail.
            def z2_out(n, b0, bw, ps, ci, ps_off=0):
                sl = slice(b0, b0 + bw)
                psl = slice(b0 + ps_off, b0 + ps_off + bw)
                zm_t = work.tile([128, bw], f16, name="zm_t", tag="zm_t")
                nc.scalar.activation(zm_t[:], ps[:, psl], AF.Sigmoid, scale=-INV)
                o_t = work.tile([128, bw], f16, name="o_t", tag="o_t")
                nc.vector.tensor_mul(o_t[:], zm_t[:], d_sb[n - 2][:, sl])
                eng = [nc.sync, nc.scalar, nc.gpsimd, nc.sync][ci]
                eng.dma_start(dh_d[n][:, sl], o_t[:])

            for n in range(2, NT - 1):
                ps = psum.tile([128, BC], f32, name="ps_z2", tag="ps")
                gate_mms(ps, n, None, wz8_sb, wz_sb, NZ_F8, h8_sb, h16_sb)
                for ci, (b0, bw) in enumerate(bhalves):
                    z2_out(n, b0, bw, ps, 0)
            # last tile runs half-major (each 512-half fully accumulated in
            # turn) so half 0's sigmoid/mul/DMA overlap half 1's matmuls, and
            # in 256-wide chunks on alternating engines to shorten the tail
            n = NT - 1
            for hi, (b0, bw) in enumerate(bhalves):
                psh = psum.tile([128, bw], f32, name=f"ps_z3{hi}", tag="ps")
                gate_mms(psh, n, None, wz8_sb, wz_sb, NZ_F8, h8_sb, h16_sb,
                         halves=[(b0, bw)], ps_off=-b0)
                if hi == 0:
                    for j in range(2):
                        z2_out(n, b0 + j * 256, 256, psh, j, ps_off=-b0)
                else:
                    # taper the final chunks so the post-matmul chain is short
                    for ci, (c0, cw) in enumerate([(0, 256), (256, 128), (384, 128)]):
                        z2_out(n, b0 + c0, cw, psh, [2, 3, 0][ci], ps_off=-b0)

    nc.compile()
    return nc


def _pack_weights(W_r, W_z, W_h):
    """Host-side packing: transpose, scale by WS=1024, split x/h parts.

    fp16/fp8 casts are value-exact for the power-of-2 scale; fp8 parts are
    clipped to +-240 (TRN FP8_EXP4 max normal).
    """

    def xpart16(W):            # [128, H] fp16: (p, m) = W[m, p] * WS
        return np.ascontiguousarray(W[:, :I].T * WS).astype(_F16)

    def hpart8(W):             # [128, 8, H] fp8: (p, k, m) = W[m, I+128k+p]*WS
        w = np.ascontiguousarray(W[:, I:].T * WS)       # [1024 kh, 1024 m]
        w = w.reshape(8, 128, H).transpose(1, 0, 2)     # [p, k, m]
        return np.clip(np.ascontiguousarray(w), -240.0, 240.0).astype(_F8)

    wz = np.ascontiguousarray(W_z.T * WS)               # [1152, 1024]
    wz = wz.reshape(9, 128, H).transpose(1, 0, 2)       # [p, k(x first), m]
    wz16 = np.ascontiguousarray(wz).astype(_F16)

    wr8 = hpart8(W_r)                                   # [128, 8, 1024]
    wr8c = np.ascontiguousarray(                        # [4, 128, 8, 256]
        wr8.reshape(128, 8, 4, 256).transpose(2, 0, 1, 3)
    )

    return {
        "wrx": xpart16(W_r),
        "wr8": wr8c,
        "wz": wz16,
        "wz8": np.ascontiguousarray(hpart8(W_z)[:, 0:2, :]),
        "whx": xpart16(W_h),
        "wh8": hpart8(W_h),
    }


def _prep_core_inputs(x, h, wpacked):
    """Per-core in_maps. x:[B,I] f32, h:[B,H] f32; weights pre-packed."""
    maps = []
    for c in range(NCORES):
        s = slice(c * BC, (c + 1) * BC)
        xT = np.ascontiguousarray(x[s].T).astype(_F16)           # [128, BC]
        hT = np.ascontiguousarray(h[s].T)                        # [H, BC] f32
        hTk = hT.reshape(8, 128, BC).transpose(1, 0, 2)          # [p, k, b]
        hTk = np.ascontiguousarray(hTk)
        m = {
            "xT": xT,
            "hT16": hTk.astype(_F16),
            "hT8": np.clip(hTk, -240.0, 240.0).astype(_F8),
        }
        m.update(wpacked)
        maps.append(m)
    return maps


def _ensure_axon_hooks_importable():
    """bass_utils imports antenv.axon_hooks when tracing is requested; some
    images ship an antenv stub without it. Provide a no-op fallback so a
    stray BASS_TRACE env var can't crash the run."""
    import sys

    try:
        import antenv.axon_hooks  # noqa: F401
    except ImportError:
        import types

        mod = types.ModuleType("antenv.axon_hooks")
        mod.get_axon_ntff_profile_hook = lambda: None
        mod.set_axon_ntff_profile_hook = lambda h: None
        sys.modules["antenv.axon_hooks"] = mod


def kernel(t, h, x_coeffs, W_r, W_z, W_h):
    _ensure_axon_hooks_importable()
    from concourse.bass_utils import run_bass_kernel_spmd

    t = np.asarray(t)
    h = np.asarray(h, dtype=np.float32)
    x_coeffs = np.asarray(x_coeffs)
    W_r = np.asarray(W_r, dtype=np.float32)
    W_z = np.asarray(W_z, dtype=np.float32)
    W_h = np.asarray(W_h, dtype=np.float32)

    t_int = int(np.clip(np.int32(float(t)), 0, x_coeffs.shape[0] - 1))
    x = np.asarray(x_coeffs[t_int], dtype=np.float32)            # [B, I]

    if "nc" not in _cache:
        _cache["nc"] = _build_nc()
    nc = _cache["nc"]

    wpacked = _pack_weights(W_r, W_z, W_h)
    in_maps = _prep_core_inputs(x, h, wpacked)

    import os

    trace = bool(os.environ.get("BASS_TRACE"))
    res = run_bass_kernel_spmd(nc, in_maps, list(range(NCORES)), trace=trace)
    _cache["last_result"] = res

    outs = []
    for c in range(NCORES):
        dhT = res.results[c]["dhT"]                              # [8,128,BC]
        outs.append(np.asarray(dhT, dtype=np.float32).reshape(H, BC))
    dhT_full = np.concatenate(outs, axis=1)                      # [H, B]
    return np.ascontiguousarray(dhT_full.T).astype(np.float32)   # [B, H]


# revision 25
# speedup vs baseline: 1.0058x; 1.0058x over previous
"""GRU-ODE delay cell on 8 Trainium2 NeuronCores (Bass/Tile), fp8 DoubleRow.

Math (per reference):
    x   = x_coeffs[int(t)]                  # [B, I]
    r   = sigmoid([x, h] @ W_r.T)
    z   = sigmoid([x, h] @ W_z.T)
    h~  = tanh([x, r*h] @ W_h.T)
    dh  = (1 - z) * (h~ - h)

Strategy: data-parallel over batch (B=8192 -> 1024 rows/core), weights
replicated, transposed ([feature, batch]) layout throughout.

Precision plan (validated against the reference in fp64 sim):
  - r gate h-part and h~ gate rh-part run as fp8e4 DoubleRow matmuls
    (2 contraction rows per PE cell -> ~1.8x matmul throughput).
  - z gate and all x-parts run in fp16 (same PE speed as bf16, 10-bit
    mantissa): dh = (1-z)(h~-h) amplifies z errors by |h~-h| (up to ~6),
    so z cannot take fp8; fp16 makes its error negligible.
  - all weights are pre-scaled by 1024 on host; activations keep natural
    scale; every PSUM readout applies scale=1/1024 inside the ACT op.
    (fp8e4 min normal is 2^-6: scaling weights up moves their mass out
    of the subnormal range.)
  - final (h~ - h) subtraction uses fp32 h.
  Simulated max-rel-err 0.0158 vs tolerance 2e-2.

Orientation per core (hidden tile m of 128 rows, batch free dim 1024):
    psum[m, b] += W.T[k_tile, m_slice].T @ act.T[k_tile, b]
    fp16 stages: one 128-row k-subtile per matmul
    fp8 stages:  DoubleRow pair = 2 k-subtiles per matmul via 3D AP
                 [128, 2, cols]
"""

import numpy as np
import ml_dtypes

B, H, I, TMAX = 8192, 1024, 128, 128
NCORES = 8
BC = B // NCORES          # batch rows per core
NT = H // 128             # 8 hidden output tiles
MM_N = 512                # moving free-dim per matmul (one PSUM bank of fp32)
WS = 1024.0               # host-side weight pre-scale (exact power of 2)

# per-gate count of h-side k-subtiles (of 8) computed in fp8 DoubleRow;
# must be even. Rest (and the x subtile) run fp16.
NR_F8 = 8
NZ_F8 = 2
NH_F8 = 8

_F16 = np.float16
_F8 = ml_dtypes.float8_e4m3   # IEEE-ish variant, max +-240 == TRN FP8_EXP4

_cache = {}


def _build_nc():
    import concourse.bacc as bacc
    import concourse.tile as tile
    import concourse.mybir as mybir

    f32 = mybir.dt.float32
    f16 = mybir.dt.float16
    f8 = mybir.dt.float8e4
    AF = mybir.ActivationFunctionType
    DR = mybir.MatmulPerfMode.DoubleRow
    INV = 1.0 / WS

    nc = bacc.Bacc(
        "TRN2",
        target_bir_lowering=False,
        debug=False,
        enable_asserts=False,
        num_devices=NCORES,
    )

    # DRAM layouts mirror the SBUF tile shapes exactly (host pre-packs).
    xT_d = nc.dram_tensor("xT", [128, BC], f16, kind="ExternalInput").ap()
    h16_d = nc.dram_tensor("hT16", [128, 8, BC], f16, kind="ExternalInput").ap()
    h8_d = nc.dram_tensor("hT8", [128, 8, BC], f8, kind="ExternalInput").ap()
    wrx_d = nc.dram_tensor("wrx", [128, H], f16, kind="ExternalInput").ap()
    # wr8 chunked by output-column group (chunk c = all 8 k-subtiles for 256
    # consecutive gate columns) so r matmuls can start after one chunk.
    wr8_d = nc.dram_tensor("wr8", [4, 128, 8, 256], f8, kind="ExternalInput").ap()
    wz_d = nc.dram_tensor("wz", [128, 9, H], f16, kind="ExternalInput").ap()
    wz8_d = nc.dram_tensor("wz8", [128, 2, H], f8, kind="ExternalInput").ap()
    whx_d = nc.dram_tensor("whx", [128, H], f16, kind="ExternalInput").ap()
    wh8_d = nc.dram_tensor("wh8", [128, 8, H], f8, kind="ExternalInput").ap()
    dh_d = nc.dram_tensor("dhT", [NT, 128, BC], f16, kind="ExternalOutput").ap()
    # sink for the PE warm-up matmuls (keeps them from being DCE'd)
    warm_d = nc.dram_tensor("warm", [128, 4], f32, kind="ExternalOutput").ap()

    bhalves = [(j * MM_N, MM_N) for j in range(BC // MM_N)]

    with tile.TileContext(nc) as tc:
        with (
            tc.tile_pool(name="res", bufs=1) as res,
            tc.tile_pool(name="work", bufs=3) as work,
            tc.tile_pool(name="psum", bufs=4, space="PSUM") as psum,
        ):
            # ---- PE warm-up input (memset must precede the warm matmuls) ----
            warm_in = res.tile([128, 512], f16, name="warm_in", tag="warm_in")
            nc.vector.memset(warm_in[:], 0.0)

            # ---- resident loads, issue-ordered by first use. dma_start
            # descriptor generation costs ~0.65us on the ISSUING engine and
            # serializes per engine; concurrent transfers share the ~330GB/s
            # DMA fabric. The r-gate critical prefix (x, wrx, h8, wr8 chunks)
            # goes on sync alone; everything else is interleaved into the
            # scalar engine's program between r-tile activations so its
            # transfers don't steal bandwidth from the prefix. ----
            x_sb = res.tile([128, BC], f16, name="x_sb", tag="x_sb")
            wrx_sb = res.tile([128, H], f16, name="wrx_sb", tag="wrx_sb")
            wr8_sb = [
                res.tile([128, 8, 256], f8, name=f"wr8_{c}", tag=f"wr8_{c}")
                for c in range(4)
            ]
            h8_sb = res.tile([128, 8, BC], f8, name="h8_sb", tag="h8_sb")
            h16_sb = res.tile([128, 8, BC], f16, name="h16_sb", tag="h16_sb")
            wz_sb = res.tile([128, 9, H], f16, name="wz_sb", tag="wz_sb")
            wz8_sb = res.tile([128, 2, H], f8, name="wz8_sb", tag="wz8_sb")
            whx_sb = res.tile([128, H], f16, name="whx_sb", tag="whx_sb")
            wh8_sb = res.tile([128, 8, H], f8, name="wh8_sb", tag="wh8_sb")

            # critical prefix split across sync/scalar/gpsimd so the
            # ~0.65us per-descriptor issue cost is paid in parallel
            nc.sync.dma_start(x_sb[:], xT_d[:])
            nc.sync.dma_start(wrx_sb[:], wrx_d[:])
            for c in range(4):
                nc.sync.dma_start(wr8_sb[c][:], wr8_d[c])
            nc.gpsimd.dma_start(h8_sb[:, 0:2, :], h8_d[:, 0:2, :])
            nc.gpsimd.dma_start(h8_sb[:, 2:4, :], h8_d[:, 2:4, :])
            nc.gpsimd.dma_start(h8_sb[:, 4:6, :], h8_d[:, 4:6, :])
            nc.gpsimd.dma_start(h8_sb[:, 6:8, :], h8_d[:, 6:8, :])

            # ---- PE warm-up: keep the PE busy from t0 so the HAM clock
            # gate reaches 2.4 GHz before the first real matmul. ~13 warm-ups
            # bridge the ~8.5us DMA latency of the first loads. The
            # warm output DMA sits on gpsimd BEHIND the h8 issues so it can't
            # delay them.
            warm_ps = psum.tile([128, 512], f32, name="warm_ps", tag="ps")
            for _ in range(26):
                nc.tensor.matmul(
                    warm_ps[:], warm_in[:, :128], warm_in[:], start=True, stop=True
                )
            warm_sb = res.tile([128, 4], f32, name="warm_sb", tag="warm_sb")
            nc.vector.tensor_copy(warm_sb[:], warm_ps[:, :4])
            nc.gpsimd.dma_start(warm_d[:], warm_sb[:])

            # late loads, interleaved into the scalar program per r tile:
            # h16 (rh muls + z moving), wz (~22us in), wh (~28us in)
            late_loads = [
                [(h16_sb[:, 0:2, :], h16_d[:, 0:2, :]),
                 (h16_sb[:, 2:4, :], h16_d[:, 2:4, :])],
                [(h16_sb[:, 4:6, :], h16_d[:, 4:6, :]),
                 (h16_sb[:, 6:8, :], h16_d[:, 6:8, :]),
                 (whx_sb[:], whx_d[:])],
                [(wz8_sb[:], wz8_d[:]),
                 (wz_sb[:, 0:5, :], wz_d[:, 0:5, :])],
                [(wh8_sb[:], wh8_d[:]),
                 (wz_sb[:, 5:9, :], wz_d[:, 5:9, :])],
                [], [], [], [],
            ]

            rh8_sb = res.tile([128, 8, BC], f8, name="rh8_sb", tag="rh8_sb")
            rh16_sb = None
            if NH_F8 < 8:
                rh16_sb = res.tile(
                    [128, 8 - NH_F8, BC], f16, name="rh16_sb", tag="rh16_sb"
                )
            # zm persists only for the two z tiles computed before the h gate
            zm_sb = [
                res.tile([128, BC], f16, name=f"zm{k}", tag=f"zm{k}")
                for k in range(2)
            ]
            # d = (h~ - h) persists for tiles whose z gate runs last
            d_sb = [
                res.tile([128, BC], f16, name=f"d{k}", tag=f"d{k}")
                for k in range(2, NT)
            ]

            def gate_x(ps, n, wx, wz16, halves=None, ps_off=0):
                """x-part stage (fp16, always first -> start=True)."""
                cols = slice(n * 128, (n + 1) * 128)
                lhsT = wx[:, cols] if wx is not None else wz16[:, 0, cols]
                for b0, bw in halves or bhalves:
                    nc.tensor.matmul(
                        ps[:, b0 + ps_off : b0 + ps_off + bw],
                        lhsT,
                        x_sb[:, b0 : b0 + bw],
                        start=True,
                        stop=False,
                    )

            def gate_h(ps, n, w8, wz16, nf8, rhs8, rhs16, rhs16_off=0,
                       halves=None, ps_off=0):
                """h-part stages: nf8 k-subtiles as fp8 DoubleRow pairs,
                the rest fp16. Emitted after gate_x (start=False)."""
                cols = slice(n * 128, (n + 1) * 128)
                nstage = nf8 // 2 + (8 - nf8)
                stage = 0
                for p in range(nf8 // 2):
                    kk = slice(2 * p, 2 * p + 2)
                    stage += 1
                    if isinstance(w8, list):
                        off = (n % 2) * 128
                        lhsT8 = w8[n // 2][:, kk, off : off + 128]
                    else:
                        lhsT8 = w8[:, kk, cols]
                    for b0, bw in halves or bhalves:
                        nc.tensor.matmul(
                            ps[:, b0 + ps_off : b0 + ps_off + bw],
                            lhsT8,
                            rhs8[:, kk, b0 : b0 + bw],
                            start=False,
                            stop=(stage == nstage),
                            perf_mode=DR,
                        )
                for k in range(nf8, 8):
                    stage += 1
                    lhsT = wz16[:, k + 1, cols]
                    rhs = rhs16[:, k - rhs16_off, :]
                    for b0, bw in halves or bhalves:
                        nc.tensor.matmul(
                            ps[:, b0 + ps_off : b0 + ps_off + bw],
                            lhsT,
                            rhs[:, b0 : b0 + bw],
                            start=False,
                            stop=(stage == nstage),
                        )

            def gate_mms(ps, n, wx, w8, wz16, nf8, rhs8, rhs16, rhs16_off=0,
                         halves=None, ps_off=0):
                gate_x(ps, n, wx, wz16, halves, ps_off)
                gate_h(ps, n, w8, wz16, nf8, rhs8, rhs16, rhs16_off, halves,
                       ps_off)

            # ---- r gate ----
            # x-stages of the first 4 tiles run up front: x lands ~2us before
            # the first fp8 chunks, so this gives the PE ~1.7us of real work
            # to chew on while h8/wr8 stream in.
            ps_r = {}
            for n in range(4):
                ps_r[n] = psum.tile([128, BC], f32, name="ps_r", tag="ps")
                gate_x(ps_r[n], n, wrx_sb, None)
            for n in range(NT):
                if n in ps_r:
                    ps = ps_r[n]
                    gate_h(ps, n, wr8_sb, None, NR_F8, h8_sb, h16_sb)
                else:
                    ps = psum.tile([128, BC], f32, name="ps_r", tag="ps")
                    gate_mms(ps, n, wrx_sb, wr8_sb, None, NR_F8, h8_sb, h16_sb)
                r_t = work.tile([128, BC], f16, name="r_t", tag="r_t")
                nc.scalar.activation(r_t[:], ps[:], AF.Sigmoid, scale=INV)
                for dst, src in late_loads[n]:
                    nc.scalar.dma_start(dst, src)
                # rh = r * h (fp8 for DoubleRow stages, fp16 for the rest)
                if NH_F8 > 0:
                    nc.vector.tensor_mul(
                        rh8_sb[:, n, :], r_t[:], h16_sb[:, n, :]
                    )
                if NH_F8 < 8 and n >= NH_F8:
                    nc.vector.tensor_mul(
                        rh16_sb[:, n - NH_F8, :], r_t[:], h16_sb[:, n, :]
                    )

            # ---- z gate, first two tiles (store zm = 1 - z = sigmoid(-pre)),
            # giving the scalar/vector engines time to finish rh[7] ----
            for n in range(2):
                ps = psum.tile([128, BC], f32, name="ps_z", tag="ps")
                gate_mms(ps, n, None, wz8_sb, wz_sb, NZ_F8, h8_sb, h16_sb)
                nc.scalar.activation(zm_sb[n][:], ps[:], AF.Sigmoid, scale=-INV)

            # ---- candidate gate ----
            for n in range(NT):
                ps = psum.tile([128, BC], f32, name="ps_h", tag="ps")
                gate_mms(
                    ps, n, whx_sb, wh8_sb, wz_sb, NH_F8, rh8_sb, rh16_sb,
                    rhs16_off=NH_F8,
                )
                for b0, bw in bhalves:
                    sl = slice(b0, b0 + bw)
                    ht = work.tile([128, bw], f16, name="ht", tag="ht")
                    nc.scalar.activation(ht[:], ps[:, sl], AF.Tanh, scale=INV)
                    if n < 2:
                        # z already known: finish dh = zm * (h~ - h) now
                        d_t = work.tile([128, bw], f16, name="d_t", tag="d_t")
                        nc.vector.tensor_sub(d_t[:], ht[:], h16_sb[:, n, sl])
                        o_t = work.tile([128, bw], f16, name="o_t", tag="o_t")
                        nc.vector.tensor_mul(o_t[:], d_t[:], zm_sb[n][:, sl])
                        nc.sync.dma_start(dh_d[n][:, sl], o_t[:])
                    else:
                        # stash h~ - h; z for this tile is computed afterwards
                        nc.vector.tensor_sub(
                            d_sb[n - 2][:, sl], ht[:], h16_sb[:, n, sl]
                        )

            # ---- z gate, remaining tiles + output ----
            # ends the kernel on the short chain sigmoid -> mul -> DMA;
            # the final tile runs in 256-wide chunks to shorten the tail.
            def z2_out(n, b0, bw, ps, ci, ps_off=0):
                sl = slice(b0, b0 + bw)
                psl = slice(b0 + ps_off, b0 + ps_off + bw)
                zm_t = work.tile([128, bw], f16, name="zm_t", tag="zm_t")
                nc.scalar.activation(zm_t[:], ps[:, psl], AF.Sigmoid, scale=-INV)
                o_t = work.tile([128, bw], f16, name="o_t", tag="o_t")
                nc.vector.tensor_mul(o_t[:], zm_t[:], d_sb[n - 2][:, sl])
                eng = [nc.sync, nc.scalar, nc.gpsimd, nc.sync][ci]
                eng.dma_start(dh_d[n][:, sl], o_t[:])

            for n in range(2, NT - 1):
                ps = psum.tile([128, BC], f32, name="ps_z2", tag="ps")
                gate_mms(ps, n, None, wz8_sb, wz_sb, NZ_F8, h8_sb, h16_sb)
                for ci, (b0, bw) in enumerate(bhalves):
                    z2_out(n, b0, bw, ps, 0)
            # last tile runs half-major (each 512-half fully accumulated in
            # turn) so half 0's sigmoid/mul/DMA overlap half 1's matmuls, and
            # in 256-wide chunks on alternating engines to shorten the tail
            n = NT - 1
            for hi, (b0, bw) in enumerate(bhalves):
                psh = psum.tile([128, bw], f32, name=f"ps_z3{hi}", tag="ps")
                gate_mms(psh, n, None, wz8_sb, wz_sb, NZ_F8, h8_sb, h16_sb,
                         halves=[(b0, bw)], ps_off=-b0)
                if hi == 0:
                    for j in range(2):
                        z2_out(n, b0 + j * 256, 256, psh, j, ps_off=-b0)
                else:
                    # taper the final chunks so the post-matmul chain is short
                    for ci, (c0, cw) in enumerate([(0, 256), (256, 128), (384, 128)]):
                        z2_out(n, b0 + c0, cw, psh, [2, 3, 0][ci], ps_off=-b0)

    nc.compile()
    return nc


def _pack_weights(W_r, W_z, W_h):
    """Host-side packing: transpose, scale by WS=1024, split x/h parts.

    fp16/fp8 casts are value-exact for the power-of-2 scale; fp8 parts are
    clipped to +-240 (TRN FP8_EXP4 max normal).
    """

    def xpart16(W):            # [128, H] fp16: (p, m) = W[m, p] * WS
        return np.ascontiguousarray(W[:, :I].T * WS).astype(_F16)

    def hpart8(W):             # [128, 8, H] fp8: (p, k, m) = W[m, I+128k+p]*WS
        w = np.ascontiguousarray(W[:, I:].T * WS)       # [1024 kh, 1024 m]
        w = w.reshape(8, 128, H).transpose(1, 0, 2)     # [p, k, m]
        return np.clip(np.ascontiguousarray(w), -240.0, 240.0).astype(_F8)

    wz = np.ascontiguousarray(W_z.T * WS)               # [1152, 1024]
    wz = wz.reshape(9, 128, H).transpose(1, 0, 2)       # [p, k(x first), m]
    wz16 = np.ascontiguousarray(wz).astype(_F16)

    wr8 = hpart8(W_r)                                   # [128, 8, 1024]
    wr8c = np.ascontiguousarray(                        # [4, 128, 8, 256]
        wr8.reshape(128, 8, 4, 256).transpose(2, 0, 1, 3)
    )

    return {
        "wrx": xpart16(W_r),
        "wr8": wr8c,
        "wz": wz16,
        "wz8": np.ascontiguousarray(hpart8(W_z)[:, 0:2, :]),
        "whx": xpart16(W_h),
        "wh8": hpart8(W_h),
    }


def _prep_core_inputs(x, h, wpacked):
    """Per-core in_maps. x:[B,I] f32, h:[B,H] f32; weights pre-packed."""
    maps = []
    for c in range(NCORES):
        s = slice(c * BC, (c + 1) * BC)
        xT = np.ascontiguousarray(x[s].T).astype(_F16)           # [128, BC]
        hT = np.ascontiguousarray(h[s].T)                        # [H, BC] f32
        hTk = hT.reshape(8, 128, BC).transpose(1, 0, 2)          # [p, k, b]
        hTk = np.ascontiguousarray(hTk)
        m = {
            "xT": xT,
            "hT16": hTk.astype(_F16),
            "hT8": np.clip(hTk, -240.0, 240.0).astype(_F8),
        }
        m.update(wpacked)
        maps.append(m)
    return maps


def _ensure_axon_hooks_importable():
    """bass_utils imports antenv.axon_hooks when tracing is requested; some
    images ship an antenv stub without it. Provide a no-op fallback so a
    stray BASS_TRACE env var can't crash the run."""
    import sys

    try:
        import antenv.axon_hooks  # noqa: F401
    except ImportError:
        import types

        mod = types.ModuleType("antenv.axon_hooks")
        mod.get_axon_ntff_profile_hook = lambda: None
        mod.set_axon_ntff_profile_hook = lambda h: None
        sys.modules["antenv.axon_hooks"] = mod


def kernel(t, h, x_coeffs, W_r, W_z, W_h):
    _ensure_axon_hooks_importable()
    from concourse.bass_utils import run_bass_kernel_spmd

    t = np.asarray(t)
    h = np.asarray(h, dtype=np.float32)
    x_coeffs = np.asarray(x_coeffs)
    W_r = np.asarray(W_r, dtype=np.float32)
    W_z = np.asarray(W_z, dtype=np.float32)
    W_h = np.asarray(W_h, dtype=np.float32)

    t_int = int(np.clip(np.int32(float(t)), 0, x_coeffs.shape[0] - 1))
    x = np.asarray(x_coeffs[t_int], dtype=np.float32)            # [B, I]

    if "nc" not in _cache:
        _cache["nc"] = _build_nc()
    nc = _cache["nc"]

    wpacked = _pack_weights(W_r, W_z, W_h)
    in_maps = _prep_core_inputs(x, h, wpacked)

    import os

    trace = bool(os.environ.get("BASS_TRACE"))
    res = run_bass_kernel_spmd(nc, in_maps, list(range(NCORES)), trace=trace)
    _cache["last_result"] = res

    outs = []
    for c in range(NCORES):
        dhT = res.results[c]["dhT"]                              # [8,128,BC]
        outs.append(np.asarray(dhT, dtype=np.float32).reshape(H, BC))
    dhT_full = np.concatenate(outs, axis=1)                      # [H, B]
    return np.ascontiguousarray(dhT_full.T).astype(np.float32)   # [B, H]


# revision 26
# speedup vs baseline: 1.0096x; 1.0038x over previous
"""GRU-ODE delay cell on 8 Trainium2 NeuronCores (Bass/Tile), fp8 DoubleRow.

Math (per reference):
    x   = x_coeffs[int(t)]                  # [B, I]
    r   = sigmoid([x, h] @ W_r.T)
    z   = sigmoid([x, h] @ W_z.T)
    h~  = tanh([x, r*h] @ W_h.T)
    dh  = (1 - z) * (h~ - h)

Strategy: data-parallel over batch (B=8192 -> 1024 rows/core), weights
replicated, transposed ([feature, batch]) layout throughout.

Precision plan (validated against the reference in fp64 sim):
  - r gate h-part and h~ gate rh-part run as fp8e4 DoubleRow matmuls
    (2 contraction rows per PE cell -> ~1.8x matmul throughput).
  - z gate and all x-parts run in fp16 (same PE speed as bf16, 10-bit
    mantissa): dh = (1-z)(h~-h) amplifies z errors by |h~-h| (up to ~6),
    so z cannot take fp8; fp16 makes its error negligible.
  - all weights are pre-scaled by 1024 on host; activations keep natural
    scale; every PSUM readout applies scale=1/1024 inside the ACT op.
    (fp8e4 min normal is 2^-6: scaling weights up moves their mass out
    of the subnormal range.)
  - final (h~ - h) subtraction uses fp32 h.
  Simulated max-rel-err 0.0158 vs tolerance 2e-2.

Orientation per core (hidden tile m of 128 rows, batch free dim 1024):
    psum[m, b] += W.T[k_tile, m_slice].T @ act.T[k_tile, b]
    fp16 stages: one 128-row k-subtile per matmul
    fp8 stages:  DoubleRow pair = 2 k-subtiles per matmul via 3D AP
                 [128, 2, cols]
"""

import numpy as np
import ml_dtypes

B, H, I, TMAX = 8192, 1024, 128, 128
NCORES = 8
BC = B // NCORES          # batch rows per core
NT = H // 128             # 8 hidden output tiles
MM_N = 512                # moving free-dim per matmul (one PSUM bank of fp32)
WS = 1024.0               # host-side weight pre-scale (exact power of 2)

# per-gate count of h-side k-subtiles (of 8) computed in fp8 DoubleRow;
# must be even. Rest (and the x subtile) run fp16.
NR_F8 = 8
NZ_F8 = 2
NH_F8 = 8

_F16 = np.float16
_F8 = ml_dtypes.float8_e4m3   # IEEE-ish variant, max +-240 == TRN FP8_EXP4

_cache = {}


def _build_nc():
    import concourse.bacc as bacc
    import concourse.tile as tile
    import concourse.mybir as mybir

    f32 = mybir.dt.float32
    f16 = mybir.dt.float16
    f8 = mybir.dt.float8e4
    AF = mybir.ActivationFunctionType
    DR = mybir.MatmulPerfMode.DoubleRow
    INV = 1.0 / WS

    nc = bacc.Bacc(
        "TRN2",
        target_bir_lowering=False,
        debug=False,
        enable_asserts=False,
        num_devices=NCORES,
    )

    # DRAM layouts mirror the SBUF tile shapes exactly (host pre-packs).
    xT_d = nc.dram_tensor("xT", [128, BC], f16, kind="ExternalInput").ap()
    h16_d = nc.dram_tensor("hT16", [128, 8, BC], f16, kind="ExternalInput").ap()
    h8_d = nc.dram_tensor("hT8", [128, 8, BC], f8, kind="ExternalInput").ap()
    wrx_d = nc.dram_tensor("wrx", [128, H], f16, kind="ExternalInput").ap()
    # wr8 chunked by output-column group (chunk c = all 8 k-subtiles for 256
    # consecutive gate columns) so r matmuls can start after one chunk.
    wr8_d = nc.dram_tensor("wr8", [4, 128, 8, 256], f8, kind="ExternalInput").ap()
    wz_d = nc.dram_tensor("wz", [128, 9, H], f16, kind="ExternalInput").ap()
    wz8_d = nc.dram_tensor("wz8", [128, 2, H], f8, kind="ExternalInput").ap()
    whx_d = nc.dram_tensor("whx", [128, H], f16, kind="ExternalInput").ap()
    wh8_d = nc.dram_tensor("wh8", [128, 8, H], f8, kind="ExternalInput").ap()
    dh_d = nc.dram_tensor("dhT", [NT, 128, BC], f16, kind="ExternalOutput").ap()
    # sink for the PE warm-up matmuls (keeps them from being DCE'd)
    warm_d = nc.dram_tensor("warm", [128, 4], f32, kind="ExternalOutput").ap()

    bhalves = [(j * MM_N, MM_N) for j in range(BC // MM_N)]

    with tile.TileContext(nc) as tc:
        with (
            tc.tile_pool(name="res", bufs=1) as res,
            tc.tile_pool(name="work", bufs=3) as work,
            tc.tile_pool(name="psum", bufs=4, space="PSUM") as psum,
        ):
            # ---- PE warm-up input (memset must precede the warm matmuls) ----
            warm_in = res.tile([128, 512], f16, name="warm_in", tag="warm_in")
            nc.vector.memset(warm_in[:], 0.0)

            # ---- resident loads, issue-ordered by first use. dma_start
            # descriptor generation costs ~0.65us on the ISSUING engine and
            # serializes per engine; concurrent transfers share the ~330GB/s
            # DMA fabric. The r-gate critical prefix (x, wrx, h8, wr8 chunks)
            # goes on sync alone; everything else is interleaved into the
            # scalar engine's program between r-tile activations so its
            # transfers don't steal bandwidth from the prefix. ----
            x_sb = res.tile([128, BC], f16, name="x_sb", tag="x_sb")
            wrx_sb = res.tile([128, H], f16, name="wrx_sb", tag="wrx_sb")
            wr8_sb = [
                res.tile([128, 8, 256], f8, name=f"wr8_{c}", tag=f"wr8_{c}")
                for c in range(4)
            ]
            h8_sb = res.tile([128, 8, BC], f8, name="h8_sb", tag="h8_sb")
            h16_sb = res.tile([128, 8, BC], f16, name="h16_sb", tag="h16_sb")
            wz_sb = res.tile([128, 9, H], f16, name="wz_sb", tag="wz_sb")
            wz8_sb = res.tile([128, 2, H], f8, name="wz8_sb", tag="wz8_sb")
            whx_sb = res.tile([128, H], f16, name="whx_sb", tag="whx_sb")
            wh8_sb = res.tile([128, 8, H], f8, name="wh8_sb", tag="wh8_sb")

            # critical prefix split across sync/scalar/gpsimd so the
            # ~0.65us per-descriptor issue cost is paid in parallel
            nc.sync.dma_start(x_sb[:], xT_d[:])
            nc.sync.dma_start(wrx_sb[:], wrx_d[:])
            for c in range(4):
                nc.sync.dma_start(wr8_sb[c][:], wr8_d[c])
            nc.gpsimd.dma_start(h8_sb[:, 0:2, :], h8_d[:, 0:2, :])
            nc.gpsimd.dma_start(h8_sb[:, 2:4, :], h8_d[:, 2:4, :])
            nc.gpsimd.dma_start(h8_sb[:, 4:6, :], h8_d[:, 4:6, :])
            nc.gpsimd.dma_start(h8_sb[:, 6:8, :], h8_d[:, 6:8, :])

            # ---- PE warm-up: keep the PE busy from t0 so the HAM clock
            # gate reaches 2.4 GHz before the first real matmul. ~13 warm-ups
            # bridge the ~8.5us DMA latency of the first loads. The
            # warm output DMA sits on gpsimd BEHIND the h8 issues so it can't
            # delay them.
            warm_ps = psum.tile([128, 512], f32, name="warm_ps", tag="ps")
            for _ in range(18):
                nc.tensor.matmul(
                    warm_ps[:], warm_in[:, :128], warm_in[:], start=True, stop=True
                )
            warm_sb = res.tile([128, 4], f32, name="warm_sb", tag="warm_sb")
            nc.vector.tensor_copy(warm_sb[:], warm_ps[:, :4])
            nc.gpsimd.dma_start(warm_d[:], warm_sb[:])

            # late loads, interleaved into the scalar program per r tile:
            # h16 (rh muls + z moving), wz (~22us in), wh (~28us in)
            late_loads = [
                [(h16_sb[:, 0:2, :], h16_d[:, 0:2, :]),
                 (h16_sb[:, 2:4, :], h16_d[:, 2:4, :])],
                [(h16_sb[:, 4:6, :], h16_d[:, 4:6, :]),
                 (h16_sb[:, 6:8, :], h16_d[:, 6:8, :]),
                 (whx_sb[:], whx_d[:])],
                [(wz8_sb[:], wz8_d[:]),
                 (wz_sb[:, 0:5, :], wz_d[:, 0:5, :])],
                [(wh8_sb[:], wh8_d[:]),
                 (wz_sb[:, 5:9, :], wz_d[:, 5:9, :])],
                [], [], [], [],
            ]

            rh8_sb = res.tile([128, 8, BC], f8, name="rh8_sb", tag="rh8_sb")
            rh16_sb = None
            if NH_F8 < 8:
                rh16_sb = res.tile(
                    [128, 8 - NH_F8, BC], f16, name="rh16_sb", tag="rh16_sb"
                )
            # zm persists only for the two z tiles computed before the h gate
            zm_sb = [
                res.tile([128, BC], f16, name=f"zm{k}", tag=f"zm{k}")
                for k in range(2)
            ]
            # d = (h~ - h) persists for tiles whose z gate runs last
            d_sb = [
                res.tile([128, BC], f16, name=f"d{k}", tag=f"d{k}")
                for k in range(2, NT)
            ]

            def gate_x(ps, n, wx, wz16, halves=None, ps_off=0):
                """x-part stage (fp16, always first -> start=True)."""
                cols = slice(n * 128, (n + 1) * 128)
                lhsT = wx[:, cols] if wx is not None else wz16[:, 0, cols]
                for b0, bw in halves or bhalves:
                    nc.tensor.matmul(
                        ps[:, b0 + ps_off : b0 + ps_off + bw],
                        lhsT,
                        x_sb[:, b0 : b0 + bw],
                        start=True,
                        stop=False,
                    )

            def gate_h(ps, n, w8, wz16, nf8, rhs8, rhs16, rhs16_off=0,
                       halves=None, ps_off=0, pairs=None):
                """h-part stages: nf8 k-subtiles as fp8 DoubleRow pairs,
                the rest fp16. Emitted after gate_x (start=False). `pairs`
                restricts to a subset of DR pairs (stop only fires on the
                overall last stage)."""
                cols = slice(n * 128, (n + 1) * 128)
                nstage = nf8 // 2 + (8 - nf8)
                stage = 0
                for p in (pairs if pairs is not None else range(nf8 // 2)):
                    kk = slice(2 * p, 2 * p + 2)
                    stage = p + 1
                    if isinstance(w8, list):
                        off = (n % 2) * 128
                        lhsT8 = w8[n // 2][:, kk, off : off + 128]
                    else:
                        lhsT8 = w8[:, kk, cols]
                    for b0, bw in halves or bhalves:
                        nc.tensor.matmul(
                            ps[:, b0 + ps_off : b0 + ps_off + bw],
                            lhsT8,
                            rhs8[:, kk, b0 : b0 + bw],
                            start=False,
                            stop=(stage == nstage),
                            perf_mode=DR,
                        )
                for k in range(nf8, 8):
                    stage = nf8 // 2 + (k - nf8) + 1
                    lhsT = wz16[:, k + 1, cols]
                    rhs = rhs16[:, k - rhs16_off, :]
                    for b0, bw in halves or bhalves:
                        nc.tensor.matmul(
                            ps[:, b0 + ps_off : b0 + ps_off + bw],
                            lhsT,
                            rhs[:, b0 : b0 + bw],
                            start=False,
                            stop=(stage == nstage),
                        )

            def gate_mms(ps, n, wx, w8, wz16, nf8, rhs8, rhs16, rhs16_off=0,
                         halves=None, ps_off=0):
                gate_x(ps, n, wx, wz16, halves, ps_off)
                gate_h(ps, n, w8, wz16, nf8, rhs8, rhs16, rhs16_off, halves,
                       ps_off)

            # ---- r gate ----
            # The first 4 tiles' stages are ordered by DMA arrival, not by
            # tile: x-stages (x+wrx land first), then DR pairs 0-1 across all
            # four tiles (needs only h8[0:4] + wr8 chunks 0-1), then pairs
            # 2-3 (h8[4:8]). This keeps PE demand matched to the ~330GB/s
            # feed so no single stall exceeds the HAM idle window.
            ps_r = {}
            for n in range(4):
                ps_r[n] = psum.tile([128, BC], f32, name="ps_r", tag="ps")
                gate_x(ps_r[n], n, wrx_sb, None)
            for n in range(4):
                gate_h(ps_r[n], n, wr8_sb, None, NR_F8, h8_sb, h16_sb,
                       pairs=[0, 1])
            for n in range(4):
                gate_h(ps_r[n], n, wr8_sb, None, NR_F8, h8_sb, h16_sb,
                       pairs=[2, 3])
                r_t = work.tile([128, BC], f16, name="r_t", tag="r_t")
                nc.scalar.activation(r_t[:], ps_r[n][:], AF.Sigmoid, scale=INV)
                for dst, src in late_loads[n]:
                    nc.scalar.dma_start(dst, src)
                nc.vector.tensor_mul(rh8_sb[:, n, :], r_t[:], h16_sb[:, n, :])
            for n in range(4, NT):
                ps = psum.tile([128, BC], f32, name="ps_r", tag="ps")
                gate_mms(ps, n, wrx_sb, wr8_sb, None, NR_F8, h8_sb, h16_sb)
                r_t = work.tile([128, BC], f16, name="r_t", tag="r_t")
                nc.scalar.activation(r_t[:], ps[:], AF.Sigmoid, scale=INV)
                for dst, src in late_loads[n]:
                    nc.scalar.dma_start(dst, src)
                nc.vector.tensor_mul(rh8_sb[:, n, :], r_t[:], h16_sb[:, n, :])

            # ---- z gate, first two tiles (store zm = 1 - z = sigmoid(-pre)),
            # giving the scalar/vector engines time to finish rh[7] ----
            for n in range(2):
                ps = psum.tile([128, BC], f32, name="ps_z", tag="ps")
                gate_mms(ps, n, None, wz8_sb, wz_sb, NZ_F8, h8_sb, h16_sb)
                nc.scalar.activation(zm_sb[n][:], ps[:], AF.Sigmoid, scale=-INV)

            # ---- candidate gate ----
            for n in range(NT):
                ps = psum.tile([128, BC], f32, name="ps_h", tag="ps")
                gate_mms(
                    ps, n, whx_sb, wh8_sb, wz_sb, NH_F8, rh8_sb, rh16_sb,
                    rhs16_off=NH_F8,
                )
                for b0, bw in bhalves:
                    sl = slice(b0, b0 + bw)
                    ht = work.tile([128, bw], f16, name="ht", tag="ht")
                    nc.scalar.activation(ht[:], ps[:, sl], AF.Tanh, scale=INV)
                    if n < 2:
                        # z already known: finish dh = zm * (h~ - h) now
                        d_t = work.tile([128, bw], f16, name="d_t", tag="d_t")
                        nc.vector.tensor_sub(d_t[:], ht[:], h16_sb[:, n, sl])
                        o_t = work.tile([128, bw], f16, name="o_t", tag="o_t")
                        nc.vector.tensor_mul(o_t[:], d_t[:], zm_sb[n][:, sl])
                        nc.sync.dma_start(dh_d[n][:, sl], o_t[:])
                    else:
                        # stash h~ - h; z for this tile is computed afterwards
                        nc.vector.tensor_sub(
                            d_sb[n - 2][:, sl], ht[:], h16_sb[:, n, sl]
                        )

            # ---- z gate, remaining tiles + output ----
            # ends the kernel on the short chain sigmoid -> mul -> DMA;
            # the final tile runs in 256-wide chunks to shorten the tail.
            def z2_out(n, b0, bw, ps, ci, ps_off=0):
                sl = slice(b0, b0 + bw)
                psl = slice(b0 + ps_off, b0 + ps_off + bw)
                zm_t = work.tile([128, bw], f16, name="zm_t", tag="zm_t")
                nc.scalar.activation(zm_t[:], ps[:, psl], AF.Sigmoid, scale=-INV)
                o_t = work.tile([128, bw], f16, name="o_t", tag="o_t")
                nc.vector.tensor_mul(o_t[:], zm_t[:], d_sb[n - 2][:, sl])
                eng = [nc.sync, nc.scalar, nc.gpsimd, nc.sync][ci]
                eng.dma_start(dh_d[n][:, sl], o_t[:])

            for n in range(2, NT - 1):
                ps = psum.tile([128, BC], f32, name="ps_z2", tag="ps")
                gate_mms(ps, n, None, wz8_sb, wz_sb, NZ_F8, h8_sb, h16_sb)
                for ci, (b0, bw) in enumerate(bhalves):
                    z2_out(n, b0, bw, ps, 0)
            # last tile runs half-major (each 512-half fully accumulated in
            # turn) so half 0's sigmoid/mul/DMA overlap half 1's matmuls, and
            # in 256-wide chunks on alternating engines to shorten the tail
            n = NT - 1
            for hi, (b0, bw) in enumerate(bhalves):
                psh = psum.tile([128, bw], f32, name=f"ps_z3{hi}", tag="ps")
                gate_mms(psh, n, None, wz8_sb, wz_sb, NZ_F8, h8_sb, h16_sb,
                         halves=[(b0, bw)], ps_off=-b0)
                if hi == 0:
                    for j in range(2):
                        z2_out(n, b0 + j * 256, 256, psh, j, ps_off=-b0)
                else:
                    # taper the final chunks so the post-matmul chain is short
                    for ci, (c0, cw) in enumerate([(0, 256), (256, 128), (384, 128)]):
                        z2_out(n, b0 + c0, cw, psh, [2, 3, 0][ci], ps_off=-b0)

    nc.compile()
    return nc


def _pack_weights(W_r, W_z, W_h):
    """Host-side packing: transpose, scale by WS=1024, split x/h parts.

    fp16/fp8 casts are value-exact for the power-of-2 scale; fp8 parts are
    clipped to +-240 (TRN FP8_EXP4 max normal).
    """

    def xpart16(W):            # [128, H] fp16: (p, m) = W[m, p] * WS
        return np.ascontiguousarray(W[:, :I].T * WS).astype(_F16)

    def hpart8(W):             # [128, 8, H] fp8: (p, k, m) = W[m, I+128k+p]*WS
        w = np.ascontiguousarray(W[:, I:].T * WS)       # [1024 kh, 1024 m]
        w = w.reshape(8, 128, H).transpose(1, 0, 2)     # [p, k, m]
        return np.clip(np.ascontiguousarray(w), -240.0, 240.0).astype(_F8)

    wz = np.ascontiguousarray(W_z.T * WS)               # [1152, 1024]
    wz = wz.reshape(9, 128, H).transpose(1, 0, 2)       # [p, k(x first), m]
    wz16 = np.ascontiguousarray(wz).astype(_F16)

    wr8 = hpart8(W_r)                                   # [128, 8, 1024]
    wr8c = np.ascontiguousarray(                        # [4, 128, 8, 256]
        wr8.reshape(128, 8, 4, 256).transpose(2, 0, 1, 3)
    )

    return {
        "wrx": xpart16(W_r),
        "wr8": wr8c,
        "wz": wz16,
        "wz8": np.ascontiguousarray(hpart8(W_z)[:, 0:2, :]),
        "whx": xpart16(W_h),
        "wh8": hpart8(W_h),
    }


def _prep_core_inputs(x, h, wpacked):
    """Per-core in_maps. x:[B,I] f32, h:[B,H] f32; weights pre-packed."""
    maps = []
    for c in range(NCORES):
        s = slice(c * BC, (c + 1) * BC)
        xT = np.ascontiguousarray(x[s].T).astype(_F16)           # [128, BC]
        hT = np.ascontiguousarray(h[s].T)                        # [H, BC] f32
        hTk = hT.reshape(8, 128, BC).transpose(1, 0, 2)          # [p, k, b]
        hTk = np.ascontiguousarray(hTk)
        m = {
            "xT": xT,
            "hT16": hTk.astype(_F16),
            "hT8": np.clip(hTk, -240.0, 240.0).astype(_F8),
        }
        m.update(wpacked)
        maps.append(m)
    return maps


def _ensure_axon_hooks_importable():
    """bass_utils imports antenv.axon_hooks when tracing is requested; some
    images ship an antenv stub without it. Provide a no-op fallback so a
    stray BASS_TRACE env var can't crash the run."""
    import sys

    try:
        import antenv.axon_hooks  # noqa: F401
    except ImportError:
        import types

        mod = types.ModuleType("antenv.axon_hooks")
        mod.get_axon_ntff_profile_hook = lambda: None
        mod.set_axon_ntff_profile_hook = lambda h: None
        sys.modules["antenv.axon_hooks"] = mod


def kernel(t, h, x_coeffs, W_r, W_z, W_h):
    _ensure_axon_hooks_importable()
    from concourse.bass_utils import run_bass_kernel_spmd

    t = np.asarray(t)
    h = np.asarray(h, dtype=np.float32)
    x_coeffs = np.asarray(x_coeffs)
    W_r = np.asarray(W_r, dtype=np.float32)
    W_z = np.asarray(W_z, dtype=np.float32)
    W_h = np.asarray(W_h, dtype=np.float32)

    t_int = int(np.clip(np.int32(float(t)), 0, x_coeffs.shape[0] - 1))
    x = np.asarray(x_coeffs[t_int], dtype=np.float32)            # [B, I]

    if "nc" not in _cache:
        _cache["nc"] = _build_nc()
    nc = _cache["nc"]

    wpacked = _pack_weights(W_r, W_z, W_h)
    in_maps = _prep_core_inputs(x, h, wpacked)

    import os

    trace = bool(os.environ.get("BASS_TRACE"))
    res = run_bass_kernel_spmd(nc, in_maps, list(range(NCORES)), trace=trace)
    _cache["last_result"] = res

    outs = []
    for c in range(NCORES):
        dhT = res.results[c]["dhT"]                              # [8,128,BC]
        outs.append(np.asarray(dhT, dtype=np.float32).reshape(H, BC))
    dhT_full = np.concatenate(outs, axis=1)                      # [H, B]
    return np.ascontiguousarray(dhT_full.T).astype(np.float32)   # [B, H]


# revision 27
# speedup vs baseline: 1.0383x; 1.0284x over previous
"""GRU-ODE delay cell on 8 Trainium2 NeuronCores (Bass/Tile), fp8 DoubleRow.

Math (per reference):
    x   = x_coeffs[int(t)]                  # [B, I]
    r   = sigmoid([x, h] @ W_r.T)
    z   = sigmoid([x, h] @ W_z.T)
    h~  = tanh([x, r*h] @ W_h.T)
    dh  = (1 - z) * (h~ - h)

Strategy: data-parallel over batch (B=8192 -> 1024 rows/core), weights
replicated, transposed ([feature, batch]) layout throughout.

Precision plan (validated against the reference in fp64 sim):
  - r gate h-part, h~ gate rh-part, and the first k-pair of the z gate
    run as fp8e4 DoubleRow matmuls (2 contraction rows per PE cell ->
    ~2x matmul throughput at the same 216ns/MM issue rate).
  - the rest of the z gate and all x-parts run in fp16 (same PE speed
    as bf16, 10-bit mantissa): dh = (1-z)(h~-h) amplifies z errors by
    |h~-h| (up to ~6), so z mostly cannot take fp8; fp16 makes its
    error negligible and buys budget for the fp8 gates.
  - all weights are pre-scaled by 1024 on host; activations keep natural
    scale; every PSUM readout applies scale=1/1024 inside the ACT op.
    (fp8e4 min normal is 2^-6: scaling weights up moves their mass out
    of the subnormal range.)
  - (h~ - h) subtracts the fp16 h (2^-11 relative, negligible); output
    dh is written in fp16 and upcast on host.
  Simulated max-rel-err 0.01651 (bit-exact match with HW) vs 2e-2.

Orientation per core (hidden tile m of 128 rows, batch free dim 1024):
    psum[m, b] += W.T[k_tile, m_slice].T @ act.T[k_tile, b]
    fp16 stages: one 128-row k-subtile per matmul
    fp8 stages:  DoubleRow pair = 2 k-subtiles per matmul via 3D AP
                 [128, 2, cols]
"""

import numpy as np
import ml_dtypes

B, H, I, TMAX = 8192, 1024, 128, 128
NCORES = 8
BC = B // NCORES          # batch rows per core
NT = H // 128             # 8 hidden output tiles
MM_N = 512                # moving free-dim per matmul (one PSUM bank of fp32)
WS = 1024.0               # host-side weight pre-scale (exact power of 2)

# per-gate count of h-side k-subtiles (of 8) computed in fp8 DoubleRow;
# must be even. Rest (and the x subtile) run fp16.
NR_F8 = 8
NZ_F8 = 2
NH_F8 = 8

_F16 = np.float16
_F8 = ml_dtypes.float8_e4m3   # IEEE-ish variant, max +-240 == TRN FP8_EXP4

_cache = {}


def _build_nc():
    import concourse.bacc as bacc
    import concourse.tile as tile
    import concourse.mybir as mybir

    f32 = mybir.dt.float32
    f16 = mybir.dt.float16
    f8 = mybir.dt.float8e4
    AF = mybir.ActivationFunctionType
    DR = mybir.MatmulPerfMode.DoubleRow
    INV = 1.0 / WS

    nc = bacc.Bacc(
        "TRN2",
        target_bir_lowering=False,
        debug=False,
        enable_asserts=False,
        num_devices=NCORES,
    )

    # DRAM layouts mirror the SBUF tile shapes exactly (host pre-packs).
    xT_d = nc.dram_tensor("xT", [128, BC], f16, kind="ExternalInput").ap()
    h16_d = nc.dram_tensor("hT16", [128, 8, BC], f16, kind="ExternalInput").ap()
    h8_d = nc.dram_tensor("hT8", [128, 8, BC], f8, kind="ExternalInput").ap()
    wrx_d = nc.dram_tensor("wrx", [128, H], f16, kind="ExternalInput").ap()
    # wr8 chunked by output-column group (chunk c = all 8 k-subtiles for 256
    # consecutive gate columns) so r matmuls can start after one chunk.
    wr8_d = nc.dram_tensor("wr8", [4, 128, 8, 256], f8, kind="ExternalInput").ap()
    wz_d = nc.dram_tensor("wz", [128, 9, H], f16, kind="ExternalInput").ap()
    wz8_d = nc.dram_tensor("wz8", [128, 2, H], f8, kind="ExternalInput").ap()
    whx_d = nc.dram_tensor("whx", [128, H], f16, kind="ExternalInput").ap()
    wh8_d = nc.dram_tensor("wh8", [128, 8, H], f8, kind="ExternalInput").ap()
    dh_d = nc.dram_tensor("dhT", [NT, 128, BC], f16, kind="ExternalOutput").ap()
    # sink for the PE warm-up matmuls (keeps them from being DCE'd)
    warm_d = nc.dram_tensor("warm", [128, 4], f32, kind="ExternalOutput").ap()

    bhalves = [(j * MM_N, MM_N) for j in range(BC // MM_N)]

    with tile.TileContext(nc) as tc:
        with (
            tc.tile_pool(name="res", bufs=1) as res,
            tc.tile_pool(name="work", bufs=3) as work,
            tc.tile_pool(name="psum", bufs=4, space="PSUM") as psum,
        ):
            # ---- PE warm-up input (memset must precede the warm matmuls) ----
            warm_in = res.tile([128, 512], f16, name="warm_in", tag="warm_in")
            nc.vector.memset(warm_in[:], 0.0)

            # ---- resident loads, issue-ordered by first use. dma_start
            # descriptor generation costs ~0.65us on the ISSUING engine and
            # serializes per engine; concurrent transfers share the ~330GB/s
            # DMA fabric. The r-gate critical prefix (x, wrx, h8, wr8 chunks)
            # goes on sync alone; everything else is interleaved into the
            # scalar engine's program between r-tile activations so its
            # transfers don't steal bandwidth from the prefix. ----
            x_sb = res.tile([128, BC], f16, name="x_sb", tag="x_sb")
            wrx_sb = res.tile([128, H], f16, name="wrx_sb", tag="wrx_sb")
            wr8_sb = [
                res.tile([128, 8, 256], f8, name=f"wr8_{c}", tag=f"wr8_{c}")
                for c in range(4)
            ]
            h8_sb = res.tile([128, 8, BC], f8, name="h8_sb", tag="h8_sb")
            h16_sb = res.tile([128, 8, BC], f16, name="h16_sb", tag="h16_sb")
            wz_sb = res.tile([128, 9, H], f16, name="wz_sb", tag="wz_sb")
            wz8_sb = res.tile([128, 2, H], f8, name="wz8_sb", tag="wz8_sb")
            whx_sb = res.tile([128, H], f16, name="whx_sb", tag="whx_sb")
            wh8_sb = res.tile([128, 8, H], f8, name="wh8_sb", tag="wh8_sb")

            # critical prefix split across sync/scalar/gpsimd so the
            # ~0.65us per-descriptor issue cost is paid in parallel
            nc.sync.dma_start(x_sb[:], xT_d[:])
            nc.sync.dma_start(wrx_sb[:], wrx_d[:])
            for c in range(4):
                nc.sync.dma_start(wr8_sb[c][:], wr8_d[c])
            nc.gpsimd.dma_start(h8_sb[:, 0:2, :], h8_d[:, 0:2, :])
            nc.gpsimd.dma_start(h8_sb[:, 2:4, :], h8_d[:, 2:4, :])
            nc.gpsimd.dma_start(h8_sb[:, 4:6, :], h8_d[:, 4:6, :])
            nc.gpsimd.dma_start(h8_sb[:, 6:8, :], h8_d[:, 6:8, :])

            # ---- PE warm-up: keep the PE busy from t0 so the HAM clock
            # gate reaches 2.4 GHz before the first real matmul. ~13 warm-ups
            # bridge the ~8.5us DMA latency of the first loads. The
            # warm output DMA sits on gpsimd BEHIND the h8 issues so it can't
            # delay them.
            warm_ps = psum.tile([128, 512], f32, name="warm_ps", tag="ps")
            for _ in range(18):
                nc.tensor.matmul(
                    warm_ps[:], warm_in[:, :128], warm_in[:], start=True, stop=True
                )
            warm_sb = res.tile([128, 4], f32, name="warm_sb", tag="warm_sb")
            nc.vector.tensor_copy(warm_sb[:], warm_ps[:, :4])
            nc.gpsimd.dma_start(warm_d[:], warm_sb[:])

            # late loads, interleaved into the scalar program per r tile:
            # h16 (rh muls + z moving), wz (~22us in), wh (~28us in)
            late_loads = [
                [(h16_sb[:, 0:2, :], h16_d[:, 0:2, :]),
                 (h16_sb[:, 2:4, :], h16_d[:, 2:4, :])],
                [(h16_sb[:, 4:6, :], h16_d[:, 4:6, :]),
                 (h16_sb[:, 6:8, :], h16_d[:, 6:8, :]),
                 (whx_sb[:], whx_d[:])],
                [(wz8_sb[:], wz8_d[:]),
                 (wz_sb[:, 0:5, :], wz_d[:, 0:5, :])],
                [(wh8_sb[:], wh8_d[:]),
                 (wz_sb[:, 5:9, :], wz_d[:, 5:9, :])],
                [], [], [], [],
            ]

            rh8_sb = res.tile([128, 8, BC], f8, name="rh8_sb", tag="rh8_sb")
            rh16_sb = None
            if NH_F8 < 8:
                rh16_sb = res.tile(
                    [128, 8 - NH_F8, BC], f16, name="rh16_sb", tag="rh16_sb"
                )
            # zm persists only for the two z tiles computed before the h gate
            zm_sb = [
                res.tile([128, BC], f16, name=f"zm{k}", tag=f"zm{k}")
                for k in range(2)
            ]
            # d = (h~ - h) persists for tiles whose z gate runs last
            d_sb = [
                res.tile([128, BC], f16, name=f"d{k}", tag=f"d{k}")
                for k in range(2, NT)
            ]

            def gate_x(ps, n, wx, wz16, halves=None, ps_off=0):
                """x-part stage (fp16, always first -> start=True)."""
                cols = slice(n * 128, (n + 1) * 128)
                lhsT = wx[:, cols] if wx is not None else wz16[:, 0, cols]
                for b0, bw in halves or bhalves:
                    nc.tensor.matmul(
                        ps[:, b0 + ps_off : b0 + ps_off + bw],
                        lhsT,
                        x_sb[:, b0 : b0 + bw],
                        start=True,
                        stop=False,
                    )

            def gate_h(ps, n, w8, wz16, nf8, rhs8, rhs16, rhs16_off=0,
                       halves=None, ps_off=0, pairs=None):
                """h-part stages: nf8 k-subtiles as fp8 DoubleRow pairs,
                the rest fp16. Emitted after gate_x (start=False). `pairs`
                restricts to a subset of DR pairs (stop only fires on the
                overall last stage)."""
                cols = slice(n * 128, (n + 1) * 128)
                nstage = nf8 // 2 + (8 - nf8)
                stage = 0
                for p in (pairs if pairs is not None else range(nf8 // 2)):
                    kk = slice(2 * p, 2 * p + 2)
                    stage = p + 1
                    if isinstance(w8, list):
                        off = (n % 2) * 128
                        lhsT8 = w8[n // 2][:, kk, off : off + 128]
                    else:
                        lhsT8 = w8[:, kk, cols]
                    for b0, bw in halves or bhalves:
                        nc.tensor.matmul(
                            ps[:, b0 + ps_off : b0 + ps_off + bw],
                            lhsT8,
                            rhs8[:, kk, b0 : b0 + bw],
                            start=False,
                            stop=(stage == nstage),
                            perf_mode=DR,
                        )
                for k in range(nf8, 8):
                    stage = nf8 // 2 + (k - nf8) + 1
                    lhsT = wz16[:, k + 1, cols]
                    rhs = rhs16[:, k - rhs16_off, :]
                    for b0, bw in halves or bhalves:
                        nc.tensor.matmul(
                            ps[:, b0 + ps_off : b0 + ps_off + bw],
                            lhsT,
                            rhs[:, b0 : b0 + bw],
                            start=False,
                            stop=(stage == nstage),
                        )

            def gate_mms(ps, n, wx, w8, wz16, nf8, rhs8, rhs16, rhs16_off=0,
                         halves=None, ps_off=0):
                gate_x(ps, n, wx, wz16, halves, ps_off)
                gate_h(ps, n, w8, wz16, nf8, rhs8, rhs16, rhs16_off, halves,
                       ps_off)

            # ---- r gate ----
            # The first 4 tiles' stages are ordered by DMA arrival, not by
            # tile: x-stages (x+wrx land first), then DR pairs 0-1 across all
            # four tiles (needs only h8[0:4] + wr8 chunks 0-1), then pairs
            # 2-3 (h8[4:8]). This keeps PE demand matched to the ~330GB/s
            # feed so no single stall exceeds the HAM idle window.
            ps_r = {}
            for n in range(4):
                ps_r[n] = psum.tile([128, BC], f32, name="ps_r", tag="ps")
                gate_x(ps_r[n], n, wrx_sb, None)
            for n in range(4):
                gate_h(ps_r[n], n, wr8_sb, None, NR_F8, h8_sb, h16_sb,
                       pairs=[0, 1])
            for n in range(4):
                gate_h(ps_r[n], n, wr8_sb, None, NR_F8, h8_sb, h16_sb,
                       pairs=[2, 3])
                r_t = work.tile([128, BC], f16, name="r_t", tag="r_t")
                nc.scalar.activation(r_t[:], ps_r[n][:], AF.Sigmoid, scale=INV)
                for dst, src in late_loads[n]:
                    nc.scalar.dma_start(dst, src)
                nc.vector.tensor_mul(rh8_sb[:, n, :], r_t[:], h16_sb[:, n, :])
            for n in range(4, NT):
                ps = psum.tile([128, BC], f32, name="ps_r", tag="ps")
                gate_mms(ps, n, wrx_sb, wr8_sb, None, NR_F8, h8_sb, h16_sb)
                r_t = work.tile([128, BC], f16, name="r_t", tag="r_t")
                nc.scalar.activation(r_t[:], ps[:], AF.Sigmoid, scale=INV)
                for dst, src in late_loads[n]:
                    nc.scalar.dma_start(dst, src)
                nc.vector.tensor_mul(rh8_sb[:, n, :], r_t[:], h16_sb[:, n, :])

            # ---- z gate, first two tiles (store zm = 1 - z = sigmoid(-pre)),
            # giving the scalar/vector engines time to finish rh[7] ----
            for n in range(2):
                ps = psum.tile([128, BC], f32, name="ps_z", tag="ps")
                gate_mms(ps, n, None, wz8_sb, wz_sb, NZ_F8, h8_sb, h16_sb)
                nc.scalar.activation(zm_sb[n][:], ps[:], AF.Sigmoid, scale=-INV)

            # ---- candidate gate ----
            for n in range(NT):
                ps = psum.tile([128, BC], f32, name="ps_h", tag="ps")
                gate_mms(
                    ps, n, whx_sb, wh8_sb, wz_sb, NH_F8, rh8_sb, rh16_sb,
                    rhs16_off=NH_F8,
                )
                for b0, bw in bhalves:
                    sl = slice(b0, b0 + bw)
                    ht = work.tile([128, bw], f16, name="ht", tag="ht")
                    nc.scalar.activation(ht[:], ps[:, sl], AF.Tanh, scale=INV)
                    if n < 2:
                        # z already known: finish dh = zm * (h~ - h) now
                        d_t = work.tile([128, bw], f16, name="d_t", tag="d_t")
                        nc.vector.tensor_sub(d_t[:], ht[:], h16_sb[:, n, sl])
                        o_t = work.tile([128, bw], f16, name="o_t", tag="o_t")
                        nc.vector.tensor_mul(o_t[:], d_t[:], zm_sb[n][:, sl])
                        nc.sync.dma_start(dh_d[n][:, sl], o_t[:])
                    else:
                        # stash h~ - h; z for this tile is computed afterwards
                        nc.vector.tensor_sub(
                            d_sb[n - 2][:, sl], ht[:], h16_sb[:, n, sl]
                        )

            # ---- z gate, remaining tiles + output ----
            # ends the kernel on the short chain sigmoid -> mul -> DMA;
            # the final tile runs in 256-wide chunks to shorten the tail.
            def z2_out(n, b0, bw, ps, ci, ps_off=0):
                sl = slice(b0, b0 + bw)
                psl = slice(b0 + ps_off, b0 + ps_off + bw)
                zm_t = work.tile([128, bw], f16, name="zm_t", tag="zm_t")
                nc.scalar.activation(zm_t[:], ps[:, psl], AF.Sigmoid, scale=-INV)
                o_t = work.tile([128, bw], f16, name="o_t", tag="o_t")
                nc.vector.tensor_mul(o_t[:], zm_t[:], d_sb[n - 2][:, sl])
                eng = [nc.sync, nc.scalar, nc.gpsimd, nc.sync][ci]
                eng.dma_start(dh_d[n][:, sl], o_t[:])

            for n in range(2, NT - 1):
                ps = psum.tile([128, BC], f32, name="ps_z2", tag="ps")
                gate_mms(ps, n, None, wz8_sb, wz_sb, NZ_F8, h8_sb, h16_sb)
                for ci, (b0, bw) in enumerate(bhalves):
                    z2_out(n, b0, bw, ps, 0)
            # last tile runs half-major (each 512-half fully accumulated in
            # turn) so half 0's sigmoid/mul/DMA overlap half 1's matmuls, and
            # in 256-wide chunks on alternating engines to shorten the tail
            n = NT - 1
            for hi, (b0, bw) in enumerate(bhalves):
                psh = psum.tile([128, bw], f32, name=f"ps_z3{hi}", tag="ps")
                gate_mms(psh, n, None, wz8_sb, wz_sb, NZ_F8, h8_sb, h16_sb,
                         halves=[(b0, bw)], ps_off=-b0)
                if hi == 0:
                    for j in range(2):
                        z2_out(n, b0 + j * 256, 256, psh, j, ps_off=-b0)
                else:
                    # taper the final chunks so the post-matmul chain is short
                    for ci, (c0, cw) in enumerate([(0, 256), (256, 128), (384, 128)]):
                        z2_out(n, b0 + c0, cw, psh, [2, 3, 0][ci], ps_off=-b0)

    nc.compile()
    return nc


def _pack_weights(W_r, W_z, W_h):
    """Host-side packing: transpose, scale by WS=1024, split x/h parts.

    fp16/fp8 casts are value-exact for the power-of-2 scale; fp8 parts are
    clipped to +-240 (TRN FP8_EXP4 max normal).
    """

    def xpart16(W):            # [128, H] fp16: (p, m) = W[m, p] * WS
        return np.ascontiguousarray(W[:, :I].T * WS).astype(_F16)

    def hpart8(W):             # [128, 8, H] fp8: (p, k, m) = W[m, I+128k+p]*WS
        w = np.ascontiguousarray(W[:, I:].T * WS)       # [1024 kh, 1024 m]
        w = w.reshape(8, 128, H).transpose(1, 0, 2)     # [p, k, m]
        return np.clip(np.ascontiguousarray(w), -240.0, 240.0).astype(_F8)

    wz = np.ascontiguousarray(W_z.T * WS)               # [1152, 1024]
    wz = wz.reshape(9, 128, H).transpose(1, 0, 2)       # [p, k(x first), m]
    wz16 = np.ascontiguousarray(wz).astype(_F16)

    wr8 = hpart8(W_r)                                   # [128, 8, 1024]
    wr8c = np.ascontiguousarray(                        # [4, 128, 8, 256]
        wr8.reshape(128, 8, 4, 256).transpose(2, 0, 1, 3)
    )

    return {
        "wrx": xpart16(W_r),
        "wr8": wr8c,
        "wz": wz16,
        "wz8": np.ascontiguousarray(hpart8(W_z)[:, 0:2, :]),
        "whx": xpart16(W_h),
        "wh8": hpart8(W_h),
    }


def _prep_core_inputs(x, h, wpacked):
    """Per-core in_maps. x:[B,I] f32, h:[B,H] f32; weights pre-packed."""
    maps = []
    for c in range(NCORES):
        s = slice(c * BC, (c + 1) * BC)
        xT = np.ascontiguousarray(x[s].T).astype(_F16)           # [128, BC]
        hT = np.ascontiguousarray(h[s].T)                        # [H, BC] f32
        hTk = hT.reshape(8, 128, BC).transpose(1, 0, 2)          # [p, k, b]
        hTk = np.ascontiguousarray(hTk)
        m = {
            "xT": xT,
            "hT16": hTk.astype(_F16),
            "hT8": np.clip(hTk, -240.0, 240.0).astype(_F8),
        }
        m.update(wpacked)
        maps.append(m)
    return maps


def _ensure_axon_hooks_importable():
    """bass_utils imports antenv.axon_hooks when tracing is requested; some
    images ship an antenv stub without it. Provide a no-op fallback so a
    stray BASS_TRACE env var can't crash the run."""
    import sys

    try:
        import antenv.axon_hooks  # noqa: F401
    except ImportError:
        import types

        mod = types.ModuleType("antenv.axon_hooks")
        mod.get_axon_ntff_profile_hook = lambda: None
        mod.set_axon_ntff_profile_hook = lambda h: None
        sys.modules["antenv.axon_hooks"] = mod


def kernel(t, h, x_coeffs, W_r, W_z, W_h):
    _ensure_axon_hooks_importable()
    from concourse.bass_utils import run_bass_kernel_spmd

    t = np.asarray(t)
    h = np.asarray(h, dtype=np.float32)
    x_coeffs = np.asarray(x_coeffs)
    W_r = np.asarray(W_r, dtype=np.float32)
    W_z = np.asarray(W_z, dtype=np.float32)
    W_h = np.asarray(W_h, dtype=np.float32)

    t_int = int(np.clip(np.int32(float(t)), 0, x_coeffs.shape[0] - 1))
    x = np.asarray(x_coeffs[t_int], dtype=np.float32)            # [B, I]

    if "nc" not in _cache:
        _cache["nc"] = _build_nc()
    nc = _cache["nc"]

    wpacked = _pack_weights(W_r, W_z, W_h)
    in_maps = _prep_core_inputs(x, h, wpacked)

    import os

    trace = bool(os.environ.get("BASS_TRACE"))
    res = run_bass_kernel_spmd(nc, in_maps, list(range(NCORES)), trace=trace)
    _cache["last_result"] = res

    outs = []
    for c in range(NCORES):
        dhT = res.results[c]["dhT"]                              # [8,128,BC]
        outs.append(np.asarray(dhT, dtype=np.float32).reshape(H, BC))
    dhT_full = np.concatenate(outs, axis=1)                      # [H, B]
    return np.ascontiguousarray(dhT_full.T).astype(np.float32)   # [B, H]
